# revision 56
# baseline (speedup 1.0000x reference)
"""Trainium2 Bass kernel for the CudaNorm FastWeight DPFP transformer layer.

Sharding: batch (8) across the 8 cores; each core runs its batch's full layer:
qkvb projection, DPFP feature maps, chunked delta-rule fast-weight scan
(C=128, depth-2 Neumann solve), output projection, residual + LayerNorm.

This revision batches the per-head elementwise work across all 16 heads with
strided (3D) access patterns, computes the attention denominator via tiny
column-sum matmuls on the (underutilized) PE instead of wide masked DVE
reductions, merges the Gram/S1 matmuls, and rebalances PSUM->SBUF copies onto
the Activation engine.  DVE instruction count per chunk drops ~4x.

Self-contained: hardcodes all shapes; host-side prep rearranges weights and
builds masks/identity constants passed as extra DRAM inputs.
"""
import os
import numpy as np
import ml_dtypes

import concourse.bass as bass
import concourse.mybir as mybir
from concourse.bass_utils import run_bass_kernel_spmd
from concourse.tile import TileContext
from concourse.vector_clock import ScopedClock, VectorClock
from contextlib import ExitStack

F32 = mybir.dt.float32
BF16 = mybir.dt.bfloat16
AF = mybir.ActivationFunctionType
OP = mybir.AluOpType
AX = mybir.AxisListType

SLEN, BSZ, DM = 2048, 8, 1024
NH, DH, NROLL = 16, 64, 2
D = 2 * NROLL * DH            # 256 feature dim
C = 128                       # chunk length
NCH = SLEN // C               # 16 chunks
EPS, LN_EPS = 1e-5, 1e-5
SCALE = 1.0 / float(np.sqrt(DH))
OQKV = NH * 192               # 3072
OTOT = OQKV + NH              # 3088 (qkv + per-head b columns)
BLK = 2 * 257                 # KQ per-head block: (dc) x [K 128 | r 1 | Q 128]
NEUMANN_DEPTH = int(os.environ.get("NEUMANN_DEPTH", "2"))

# ---------------------------------------------------------------- tile ctx
MAXW = 2


class PatchedTileContext(TileContext):
    """Work around walrus TPB sync-command limits: each instruction carries at
    most 2 sync commands (waits+updates); hoist excess waits onto preceding
    same-engine NoOps (1 wait each), and emit the kernel-tail drain's waits
    one-per-nop on SP."""

    def _lower_ordered_insts(self, ordered):
        for bb_name in list(ordered.keys()):
            new = []
            for inst in ordered[bb_name]:
                si = inst.sync_info
                nupd = len(si.on_update) if si is not None and si.on_update else 0
                maxw = max(0, MAXW - nupd)
                if si is not None and si.on_wait and len(si.on_wait) > maxw:
                    waits = list(si.on_wait)
                    excess = waits if maxw == 0 else waits[:-maxw]
                    keep = [] if maxw == 0 else waits[-maxw:]
                    for w in excess:
                        nop = mybir.InstNoOp(
                            name=self.nc.get_next_instruction_name(),
                            engine=inst.engine, ins=[], outs=[])
                        nop.sync_info = mybir.SyncInfo(on_wait=[w], on_update=[])
                        new.append(nop)
                    inst.sync_info = mybir.SyncInfo(
                        on_wait=keep, on_update=list(si.on_update or []))
                new.append(inst)
            ordered[bb_name] = new
        return super()._lower_ordered_insts(ordered)

    def _drain_and_barrier(self, tick_clock, wait_clock):
        gc = tick_clock.global_clock
        n = len(gc)
        for p in range(n):
            if gc[p] > 0:
                vc = VectorClock([gc[i] if i == p else 0 for i in range(n)])
                nop = self.nc.sync.nop(nofuse=True)
                wait_clock.add_sem_waits(nop.ins, ScopedClock({None: vc}))
        self.nc.sync.drain()
        self.nc.all_engine_barrier()
        assert self.sems is not None
        popped = self.nc._tile_sem_poison_stack.pop()
        assert popped is self._sem_poison
        self.nc.clear_and_free_semaphores(list(self.sems.allocated().values()))
        self.nc.all_engine_barrier()


# ---------------------------------------------------------------- program
def build_program(n_chunks=NCH, n_heads=NH):
    nc = bass.Bass()
    d_hT = nc.declare_dram_parameter("hT", [DM, SLEN], BF16, isOutput=False)
    d_hres = nc.declare_dram_parameter("hres", [SLEN, DM], F32, isOutput=False)
    d_w = nc.declare_dram_parameter("wqkv", [DM, OTOT], BF16, isOutput=False)
    d_wo = nc.declare_dram_parameter("woT", [DM, DM], BF16, isOutput=False)
    d_lng = nc.declare_dram_parameter("lng", [128, DM], BF16, isOutput=False)
    d_lnb = nc.declare_dram_parameter("lnb", [128, DM], BF16, isOutput=False)
    d_mSL = nc.declare_dram_parameter("maskSL", [128, 132], F32, isOutput=False)
    d_mUI = nc.declare_dram_parameter("maskUI", [128, 128], F32, isOutput=False)
    d_id = nc.declare_dram_parameter("identb", [128, 128], BF16, isOutput=False)
    d_out = nc.declare_dram_parameter("out", [SLEN, DM], F32, isOutput=True)

    with PatchedTileContext(nc) as tc, ExitStack() as ctx:
        P = lambda name, bufs, **kw: ctx.enter_context(
            tc.tile_pool(name=name, bufs=bufs, **kw))
        const = P("const", 1)
        state = P("state", 1)
        fr2 = P("fr2", 2)          # front-phase tiles also read in pass 2
        fr1 = P("fr1", 1)          # front-phase scratch (dead before pass 2)
        p2 = P("p2", 1)            # pass-2 tiles
        psA_p = P("psA", 3, space="PSUM")   # proj / pass1 / dn (tag blk)
        psT_p = P("psT", 2, space="PSUM")   # transpose groups (tag tp, bf16)
        psS_p = P("psS", 2, space="PSUM")   # solve/state groups (tag s)
        psO_p = P("psO", 1, space="PSUM")   # out-proj (tag pAT)

        # ---- chunk-0 inputs first so their DMAs precede the weight bulk
        hts0 = fr2.tile([128, 1024], BF16, tag="hts", name="hts")
        nc.sync.dma_start(hts0.rearrange("p (mc t) -> p mc t", mc=8),
                          d_hT.rearrange("(mc p) t -> p mc t", p=128)[:, :, 0:128])
        hr0 = fr2.tile([128, DM], F32, tag="hr", name="hr")
        nc.sync.dma_start(hr0[:], d_hres[0:128, :])

        # ---- constants
        t_mSL = const.tile([128, 132], F32, tag="mSL", name="mSL"); nc.sync.dma_start(t_mSL[:], d_mSL[:])
        t_mUI = const.tile([128, 128], F32, tag="mUI", name="mUI"); nc.sync.dma_start(t_mUI[:], d_mUI[:])
        t_id = const.tile([128, 128], BF16, tag="id", name="id"); nc.sync.dma_start(t_id[:], d_id[:])
        t_lng = const.tile([128, DM], BF16, tag="lng", name="lng"); nc.sync.dma_start(t_lng[:], d_lng[:])
        t_lnb = const.tile([128, DM], BF16, tag="lnb", name="lnb"); nc.sync.dma_start(t_lnb[:], d_lnb[:])
        t_w = const.tile([128, 8 * OTOT], BF16, tag="w", name="w")
        for og in range(7):
            o0, ow = og * 512, (512 if og < 6 else NH)
            nc.sync.dma_start(
                t_w.rearrange("p (mc o) -> p mc o", mc=8)[:, :, o0:o0 + ow],
                d_w.rearrange("(mc p) o -> p mc o", p=128)[:, :, o0:o0 + ow])
        t_wo = const.tile([128, 8 * DM], BF16, tag="wo", name="wo")
        nc.sync.dma_start(t_wo.rearrange("p (ic o) -> p ic o", ic=8),
                          d_wo.rearrange("(ic p) o -> p ic o", p=128))
        t_ones = const.tile([128, 1], BF16, tag="ones", name="ones")
        nc.vector.memset(t_ones[:], 1.0)

        # ---- state: W [feat-in-dc 128, (h,dc) x 64], r [128, (h,dc)]
        t_Wm = state.tile([128, NH * 128], F32, tag="Wm", name="Wm")
        nc.vector.memset(t_Wm[:], 0.0)
        t_Wb = state.tile([128, NH * 128], BF16, tag="Wb", name="Wb")
        nc.vector.memset(t_Wb[:], 0.0)
        t_r = state.tile([128, 2 * NH], F32, tag="r", name="r")
        nc.vector.memset(t_r[:], 0.0)
        t_rb = state.tile([128, 2 * NH], BF16, tag="rb", name="rb")
        nc.vector.memset(t_rb[:], 0.0)

        def inject_r(nf):
            """Copy the (old) r state into KQ's per-(h,dc) column 128."""
            KQh = nf["KQ"].rearrange("p (h c) -> p h c", c=BLK)
            rbv = t_rb.rearrange("p (h d) -> p h d", d=2)
            for dc in range(2):
                nc.gpsimd.tensor_copy(KQh[:, :, dc * 257 + 128: dc * 257 + 129],
                                      rbv[:, :, dc:dc + 1])

        def front(c, hts=None, hr=None):
            cs = slice(c * 128, (c + 1) * 128)
            # ================= projection =================================
            if hts is None:
                hts = fr2.tile([128, 1024], BF16, tag="hts", name="hts")
                nc.sync.dma_start(hts.rearrange("p (mc t) -> p mc t", mc=8),
                                  d_hT.rearrange("(mc p) t -> p mc t", p=128)[:, :, cs])
                hr = fr2.tile([128, DM], F32, tag="hr", name="hr")
                nc.sync.dma_start(hr[:], d_hres[cs, :])
            raw = fr2.tile([128, 1024], BF16, tag="raw", name="raw")  # v only
            sig = fr2.tile([128, NH], F32, tag="sig", name="sig")
            # qk-first weight layout: cols h*128 = [q(h) 64 | k(h) 64] for
            # h<16, then v blocks, then b.  relu reads the projection PSUM
            # directly; relu+rolls split per og group so DVE front work
            # starts right after the first projection group.
            xp = fr1.tile([128, 2 * NH * 128], BF16, tag="xp", name="xp")
            xpv = xp.rearrange("p (b c) -> p b c", c=128)
            f_q = fr1.tile([128, NH * 256], BF16, tag="f_q", name="f_q")
            f_k = fr2.tile([128, NH * 256], BF16, tag="f_k", name="f_k")
            fs = fr1.tile([128, 32], F32, tag="fs", name="fs")
            frec = fr1.tile([128, 32], F32, tag="frec", name="frec")
            for og in range(7):
                o0, ow = og * 512, (512 if og < 6 else NH)
                pg = psA_p.tile([128, ow], F32, tag="blk", name="blk")
                for mc in range(8):
                    nc.tensor.matmul(pg[:], hts[:, mc * 128:(mc + 1) * 128],
                                     t_w[:, mc * OTOT + o0: mc * OTOT + o0 + ow],
                                     start=(mc == 0), stop=(mc == 7))
                if og < 4:
                    pgh = pg.rearrange("p (h c) -> p h c", c=128)
                    hs = slice(og * 4, og * 4 + 4)
                    nc.scalar.activation(xpv[:, hs, 0:64], pgh[:, :, 0:64], AF.Relu)
                    nc.scalar.activation(xpv[:, hs, 64:128], pgh[:, :, 0:64],
                                         AF.Relu, scale=-1.0)
                    ks = slice(16 + og * 4, 16 + og * 4 + 4)
                    nc.scalar.activation(xpv[:, ks, 0:64], pgh[:, :, 64:128], AF.Relu)
                    nc.scalar.activation(xpv[:, ks, 64:128], pgh[:, :, 64:128],
                                         AF.Relu, scale=-1.0)
                    for g, ft in ((0, f_q), (1, f_k)):
                        xv = xpv[:, g * 16 + og * 4: g * 16 + og * 4 + 4, :]
                        fv = ft.rearrange("p (h c) -> p h c",
                                          c=256)[:, og * 4: og * 4 + 4, :]
                        nc.vector.tensor_mul(fv[:, :, 1:128], xv[:, :, 1:128],
                                             xv[:, :, 0:127])
                        nc.vector.tensor_mul(fv[:, :, 0:1], xv[:, :, 0:1],
                                             xv[:, :, 127:128])
                        nc.vector.tensor_mul(fv[:, :, 130:256], xv[:, :, 2:128],
                                             xv[:, :, 0:126])
                        nc.vector.tensor_mul(fv[:, :, 128:130], xv[:, :, 0:2],
                                             xv[:, :, 126:128])
                elif og < 6:
                    nc.scalar.copy(raw[:, (og - 4) * 512:(og - 3) * 512], pg[:])
                else:
                    nc.scalar.activation(sig[:], pg[:], AF.Sigmoid)
            # normalize only the k features: the q normalization cancels in
            # out/denominator (both scale by 1/sum_q) up to the eps term
            fv = f_k.rearrange("p (h c) -> p h c", c=256)
            fold = fr1.tile([128, NH * 128], BF16, tag="fold", name="fold")
            foldv = fold.rearrange("p (h c) -> p h c", c=128)
            nc.vector.tensor_add(foldv[:], fv[:, :, 0:128], fv[:, :, 128:256])
            fold2 = fr1.tile([128, NH * 64], BF16, tag="fold2", name="fold2")
            f2v = fold2.rearrange("p (h c) -> p h c", c=64)
            nc.vector.tensor_add(f2v[:], foldv[:, :, 0:64], foldv[:, :, 64:128])
            nc.vector.tensor_reduce(fs[:, 0:16], f2v[:], AX.X, OP.add)
            nc.vector.reciprocal(frec[:, 0:16], fs[:, 0:16])
            fkv = f_k.rearrange("p (h c) -> p h c", c=256)
            nc.vector.tensor_mul(fkv[:], fkv[:],
                                 frec[:, 0:16].unsqueeze(2).broadcast_to([128, 16, 256]))

            # ================= KQ: transposed features ====================
            # KQ per (h,dc) 257-block: [K^T 128 | r 1 | Q^T 128]; the r column
            # is injected by back(c-1) after its r-state update.
            KQ = fr2.tile([128, NH * BLK], BF16, tag="KQ", name="KQ")
            KQh = KQ.rearrange("p (h c) -> p h c", c=BLK)
            for g, ft in ((0, f_q), (1, f_k)):
                for dc in range(2):
                    for hg in range(2):
                        pt = psT_p.tile([128, 1024], BF16, tag="tp", name="tp")
                        for j in range(8):
                            hd = hg * 8 + j
                            nc.tensor.transpose(
                                pt[:, j * 128:(j + 1) * 128],
                                ft[:, hd * 256 + dc * 128: hd * 256 + dc * 128 + 128],
                                t_id[:])
                        off = dc * 257 + (129 if g == 0 else 0)
                        if g == 0:
                            nc.scalar.copy(
                                KQh[:, hg * 8:(hg + 1) * 8, off:off + 128],
                                pt.rearrange("p (j c) -> p j c", j=8))
                        else:
                            nc.vector.tensor_copy(
                                KQh[:, hg * 8:(hg + 1) * 8, off:off + 128],
                                pt.rearrange("p (j c) -> p j c", j=8))
            return {"hr": hr, "raw": raw, "sig": sig, "f_k": f_k, "KQ": KQ}

        def back(c, cur, nxt):
            cs = slice(c * 128, (c + 1) * 128)
            raw, sig, f_k, KQ, hr = (cur["raw"], cur["sig"], cur["f_k"],
                                     cur["KQ"], cur["hr"])
            # ================= pass 1: Gram+S1, kd, sh =====================
            kd_all = p2.tile([128, NH], F32, tag="kd", name="kd")
            Am = p2.tile([128, NH * 132], BF16, tag="Am", name="Am")
            sh = p2.tile([128, NH * 128], BF16, tag="sh", name="sh")
            for hd in range(n_heads):
                pAS = psA_p.tile([128, 260], F32, tag="blk", name="blk")
                for dc in range(2):
                    b = hd * BLK + dc * 257
                    nc.tensor.matmul(pAS[:, 0:257], KQ[:, b:b + 128],
                                     KQ[:, b:b + 257],
                                     start=(dc == 0), stop=(dc == 1))
                nc.vector.scalar_tensor_tensor(
                    Am[:, hd * 132: hd * 132 + 129], pAS[:, 0:129], 1.0,
                    t_mSL[:, 0:129], OP.mult, OP.mult,
                    accum_out=kd_all[:, hd:hd + 1])
                nc.vector.tensor_mul(sh[:, hd * 128:(hd + 1) * 128],
                                     pAS[:, 129:257], t_mUI[:])
            if c == 0:
                nc.vector.memset(kd_all[0:1, :], 1.0)

            # dn[t] = colsum(sh)[t] + q_t . r   (tiny PE matmuls)
            p_dn = psA_p.tile([128, NH], F32, tag="blk", name="blk")
            for hd in range(n_heads):
                nc.tensor.matmul(p_dn[:, hd:hd + 1], sh[:, hd * 128:(hd + 1) * 128],
                                 t_ones[:], start=True, stop=False)
                nc.tensor.matmul(p_dn[:, hd:hd + 1],
                                 KQ[:, hd * BLK + 129: hd * BLK + 257],
                                 t_rb[:, hd * 2: hd * 2 + 1], start=False, stop=False)
                nc.tensor.matmul(p_dn[:, hd:hd + 1],
                                 KQ[:, hd * BLK + 257 + 129: hd * BLK + 257 + 257],
                                 t_rb[:, hd * 2 + 1: hd * 2 + 2], start=False, stop=True)
            dn_all = p2.tile([128, NH], F32, tag="dn", name="dn")
            nc.scalar.copy(dn_all[:], p_dn[:])

            # ================= chunk-level columns =========================
            ceps = p2.tile([128, NH], F32, tag="ceps", name="ceps")
            nc.vector.tensor_scalar_add(ceps[:], kd_all[:], EPS)
            c_all = p2.tile([128, NH], F32, tag="c", name="c")
            nc.vector.reciprocal(c_all[:], ceps[:])
            t0 = p2.tile([128, NH], F32, tag="t0", name="t0")
            nc.vector.tensor_mul(t0[:], kd_all[:], c_all[:])
            cb_all = p2.tile([128, NH], F32, tag="cb", name="cb")
            nc.vector.tensor_mul(cb_all[:], t0[:], sig[:])
            cbc = p2.tile([128, NH], F32, tag="cbc", name="cbc")
            nc.vector.tensor_mul(cbc[:], cb_all[:], c_all[:])
            ncbc = p2.tile([128, NH], F32, tag="ncbc", name="ncbc")
            nc.vector.tensor_scalar_mul(ncbc[:], cbc[:], -1.0)
            dne = p2.tile([128, NH], F32, tag="dne", name="dne")
            nc.vector.tensor_scalar_add(dne[:], dn_all[:], EPS)
            dnr = p2.tile([128, NH], F32, tag="dnr", name="dnr")
            nc.vector.reciprocal(dnr[:], dne[:])
            dnrS = p2.tile([128, NH], F32, tag="dnrS", name="dnrS")
            nc.vector.tensor_scalar_mul(dnrS[:], dnr[:], SCALE)

            # ================= pass 2 (step-major over heads) ==============
            # N'' = masked Gram * cbc (scaled in place), its transpose Bt
            Amv = Am.rearrange("p (h c) -> p h c", c=132)[:, :, 0:128]
            nc.vector.tensor_mul(Amv, Amv,
                                 cbc.unsqueeze(2).broadcast_to([128, NH, 128]))
            Bt = p2.tile([128, NH * 128], BF16, tag="Bt", name="Bt")
            for hg in range(2):
                pt = psT_p.tile([128, 1024], BF16, tag="tp", name="tp")
                for j in range(8):
                    hd = hg * 8 + j
                    nc.tensor.transpose(pt[:, j * 128:(j + 1) * 128],
                                        Am[:, hd * 132: hd * 132 + 128], t_id[:])
                nc.scalar.copy(Bt[:, hg * 1024:(hg + 1) * 1024], pt[:])

            # cbV = cb * V ; X0 = cbV - cbc * (K W)
            cbV = p2.tile([128, NH * DH], BF16, tag="cbV", name="cbV")
            for hd in range(n_heads):
                nc.scalar.mul(cbV[:, hd * 64:(hd + 1) * 64],
                              raw[:, hd * 64:(hd + 1) * 64],
                              cb_all[:, hd:hd + 1])
            if c == 0:
                X0 = cbV
            else:
                X0 = p2.tile([128, NH * DH], BF16, tag="X0", name="X0")
                pKW = []
                for hg in range(2):
                    pk = psS_p.tile([128, 512], F32, tag="s", name="s")
                    for j in range(8):
                        hd = hg * 8 + j
                        for dc in range(2):
                            nc.tensor.matmul(
                                pk[:, j * 64:(j + 1) * 64],
                                KQ[:, hd * BLK + dc * 257: hd * BLK + dc * 257 + 128],
                                t_Wb[:, hd * 128 + dc * 64: hd * 128 + dc * 64 + 64],
                                start=(dc == 0), stop=(dc == 1))
                    pKW.append(pk)
                for hg in range(2):
                    xg = X0[:, hg * 512:(hg + 1) * 512].rearrange(
                        "p (j c) -> p j c", c=64)
                    nc.vector.tensor_mul(
                        xg, pKW[hg].rearrange("p (j c) -> p j c", c=64),
                        ncbc[:, hg * 8:(hg + 1) * 8].unsqueeze(2).broadcast_to(
                            [128, 8, 64]))
                    nc.vector.tensor_add(X0[:, hg * 512:(hg + 1) * 512],
                                         X0[:, hg * 512:(hg + 1) * 512],
                                         cbV[:, hg * 512:(hg + 1) * 512])

            # Neumann solve: depth2: Y = X0 - N(X0 - N X0)
            def mmN(dst_pool_tag, rhs, hg):
                ps = psS_p.tile([128, 512], F32, tag="s", name="s")
                for j in range(8):
                    hd = hg * 8 + j
                    nc.tensor.matmul(ps[:, j * 64:(j + 1) * 64],
                                     Bt[:, hd * 128:(hd + 1) * 128],
                                     rhs[:, hd * 64:(hd + 1) * 64],
                                     start=True, stop=True)
                return ps

            if NEUMANN_DEPTH == 2:
                X1 = p2.tile([128, NH * DH], BF16, tag="X1", name="X1")
                for hg in range(2):
                    pX = mmN("s", X0, hg)
                    nc.vector.tensor_sub(X1[:, hg * 512:(hg + 1) * 512],
                                         X0[:, hg * 512:(hg + 1) * 512], pX[:])
                Yt = X0  # overwritten in place; X0 is dead after the subs
                for hg in range(2):
                    pY = mmN("s", X1, hg)
                    nc.vector.tensor_sub(Yt[:, hg * 512:(hg + 1) * 512],
                                         X0[:, hg * 512:(hg + 1) * 512], pY[:])
            else:
                # depth-3 (baseline): B2 = N N^T-form, X1 = X0 + N^2 X0,
                # Y = X1 - N X1
                B2 = p2.tile([128, NH * 128], BF16, tag="B2", name="B2")
                for hg in range(2):
                    pb2 = psS_p.tile([128, 1024], F32, tag="s2", name="s2")
                    for j in range(8):
                        hd = hg * 8 + j
                        nc.tensor.matmul(pb2[:, j * 128:(j + 1) * 128],
                                         Nt[:, hd * 128:(hd + 1) * 128],
                                         Bt[:, hd * 128:(hd + 1) * 128],
                                         start=True, stop=True)
                    nc.scalar.copy(B2[:, hg * 1024:(hg + 1) * 1024], pb2[:])
                X1 = p2.tile([128, NH * DH], BF16, tag="X1", name="X1")
                for hg in range(2):
                    ps = psS_p.tile([128, 512], F32, tag="s", name="s")
                    for j in range(8):
                        hd = hg * 8 + j
                        nc.tensor.matmul(ps[:, j * 64:(j + 1) * 64],
                                         B2[:, hd * 128:(hd + 1) * 128],
                                         X0[:, hd * 64:(hd + 1) * 64],
                                         start=True, stop=True)
                    nc.vector.tensor_add(X1[:, hg * 512:(hg + 1) * 512],
                                         X0[:, hg * 512:(hg + 1) * 512], ps[:])
                Yt = p2.tile([128, NH * DH], BF16, tag="Yt", name="Yt")
                for hg in range(2):
                    pY = mmN("s", X1, hg)
                    nc.vector.tensor_sub(Yt[:, hg * 512:(hg + 1) * 512],
                                         X1[:, hg * 512:(hg + 1) * 512], pY[:])

            # Out = QW + tril(S1) Y, scaled by SCALE/(denom+eps); transpose
            outT = p2.tile([128, NH * DH], BF16, tag="outT", name="outT")
            ptO = psT_p.tile([128, 1024], BF16, tag="tp", name="tp")
            for hg in range(2):
                po = psS_p.tile([128, 512], F32, tag="s", name="s")
                for j in range(8):
                    hd = hg * 8 + j
                    o = slice(j * 64, (j + 1) * 64)
                    if c > 0:
                        for dc in range(2):
                            nc.tensor.matmul(
                                po[:, o],
                                KQ[:, hd * BLK + dc * 257 + 129: hd * BLK + dc * 257 + 257],
                                t_Wb[:, hd * 128 + dc * 64: hd * 128 + dc * 64 + 64],
                                start=(dc == 0), stop=False)
                    nc.tensor.matmul(po[:, o], sh[:, hd * 128:(hd + 1) * 128],
                                     Yt[:, hd * 64:(hd + 1) * 64],
                                     start=(c == 0), stop=True)
                outc = p2.tile([128, 512], BF16, tag=f"outc{hg}", name=f"outc{hg}")
                for j in range(8):
                    hd = hg * 8 + j
                    nc.scalar.mul(outc[:, j * 64:(j + 1) * 64],
                                  po[:, j * 64:(j + 1) * 64], dnrS[:, hd:hd + 1])
                for j in range(8):
                    hd = hg * 8 + j
                    base = (hd % 2) * 64
                    ic = hd // 2
                    nc.tensor.transpose(ptO[base:base + 64, ic * 128:(ic + 1) * 128],
                                        outc[:, j * 64:(j + 1) * 64], t_id[:],
                                        tile_position=(0, base))
            nc.scalar.copy(outT[:], ptO[:])

            # W update: W += K^T Y (per (hg, dc) psum groups), r update
            Wmv = t_Wm.rearrange("p (h c) -> p h c", c=128)
            for hg in range(2):
                for dc in range(2):
                    pw = psS_p.tile([128, 512], F32, tag="s", name="s")
                    for j in range(8):
                        hd = hg * 8 + j
                        nc.tensor.matmul(pw[:, j * 64:(j + 1) * 64],
                                         f_k[:, hd * 256 + dc * 128: hd * 256 + dc * 128 + 128],
                                         Yt[:, hd * 64:(hd + 1) * 64],
                                         start=True, stop=True)
                    wv = Wmv[:, hg * 8:(hg + 1) * 8, dc * 64:(dc + 1) * 64]
                    nc.vector.tensor_add(wv, pw.rearrange("p (j c) -> p j c", j=8), wv)
            nc.gpsimd.tensor_copy(t_Wb[:], t_Wm[:])
            # r += per-(h,dc) row-sums of K over time, via ones-matmuls
            p_rs = psA_p.tile([128, 2 * NH], F32, tag="blk", name="blk")
            for b in range(2 * NH):
                nc.tensor.matmul(p_rs[:, b:b + 1],
                                 f_k[:, b * 128:(b + 1) * 128],
                                 t_ones[:], start=True, stop=True)
            nc.vector.tensor_add(t_r[:], t_r[:], p_rs[:])
            nc.gpsimd.tensor_copy(t_rb[:], t_r[:])
            if nxt is not None:
                inject_r(nxt)

            # ================= output projection + residual + LN ============
            x = p2.tile([128, DM], F32, tag="x", name="x")
            for og in range(2):
                pAT = psO_p.tile([128, 512], F32, tag="pAT", name="pAT")
                for ic in range(8):
                    nc.tensor.matmul(pAT[:], outT[:, ic * 128:(ic + 1) * 128],
                                     t_wo[:, ic * DM + og * 512: ic * DM + og * 512 + 512],
                                     start=(ic == 0), stop=(ic == 7))
                nc.vector.tensor_add(x[:, og * 512:(og + 1) * 512], pAT[:],
                                     hr[:, og * 512:(og + 1) * 512])
            xsum = p2.tile([128, 1], F32, tag="xsum", name="xsum")
            nc.scalar.activation(hr[:], x[:], AF.Copy, accum_out=xsum[:])
            nmu = p2.tile([128, 1], F32, tag="nmu", name="nmu")
            nc.vector.tensor_scalar_mul(nmu[:], xsum[:], -1.0 / DM)
            nc.vector.tensor_scalar_add(x[:], x[:], nmu[:])
            var = p2.tile([128, 1], F32, tag="var", name="var")
            nc.scalar.activation(hr[:], x[:], AF.Square, accum_out=var[:])
            vare = p2.tile([128, 1], F32, tag="vare", name="vare")
            nc.vector.tensor_scalar(vare[:], var[:], 1.0 / DM, float(LN_EPS),
                                    OP.mult, OP.add)
            sd = p2.tile([128, 1], F32, tag="sd", name="sd")
            nc.scalar.sqrt(sd[:], vare[:])
            rstd = p2.tile([128, 1], F32, tag="rstd", name="rstd")
            nc.vector.reciprocal(rstd[:], sd[:])
            nc.vector.scalar_tensor_tensor(x[:], x[:], rstd[:], t_lng[:],
                                           OP.mult, OP.mult)
            nc.vector.tensor_add(hr[:], x[:], t_lnb[:])
            nc.sync.dma_start(d_out[cs, :], hr[:])

        # software pipeline: emit front(c+1) ahead of back(c) so the
        # scheduler fills back(c)'s dependency stalls with front work
        tiles = {0: front(0, hts0, hr0)}
        inject_r(tiles[0])
        for c in range(n_chunks):
            if c + 1 < n_chunks:
                tiles[c + 1] = front(c + 1)
            back(c, tiles.pop(c), tiles.get(c + 1))

    return nc


# ---------------------------------------------------------------- host side
def _prep_core_inputs(h_b, W_qkvb, W_o, ln_g, ln_b):
    bf16 = ml_dtypes.bfloat16
    hT = np.ascontiguousarray(h_b.T).astype(bf16)                  # [1024, 2048]
    wq = np.zeros((DM, OTOT), dtype=bf16)
    Wr = W_qkvb.reshape(NH, 193, DM)
    for hd in range(NH):
        wq[:, hd * 128:hd * 128 + 64] = Wr[hd, 0:64, :].T        # q
        wq[:, hd * 128 + 64:hd * 128 + 128] = Wr[hd, 64:128, :].T  # k
        wq[:, 2048 + hd * 64:2048 + hd * 64 + 64] = Wr[hd, 128:192, :].T  # v
        wq[:, OQKV + hd] = Wr[hd, 192, :]
    woT = np.ascontiguousarray(W_o.T).astype(bf16)                 # [i, o]
    lng = np.broadcast_to(ln_g[None, :], (128, DM)).astype(bf16).copy()
    lnb = np.broadcast_to(ln_b[None, :], (128, DM)).astype(bf16).copy()
    ii, jj = np.indices((128, 132))
    mSL = (jj < ii).astype(np.float32);  mSL[:, 128] = 1.0
    iu, ju = np.indices((128, 128))
    mUI = (ju >= iu).astype(np.float32)
    identb = np.eye(128, dtype=bf16)
    return {"hT": hT, "hres": np.ascontiguousarray(h_b, np.float32),
            "wqkv": wq, "woT": woT, "lng": lng, "lnb": lnb,
            "maskSL": mSL, "maskUI": mUI, "identb": identb}


_cached = {}


def kernel(h, W_qkvb, W_o, ln_g, ln_b):
    h = np.asarray(h, np.float32)
    W_qkvb = np.asarray(W_qkvb, np.float32)
    W_o = np.asarray(W_o, np.float32)
    ln_g = np.asarray(ln_g, np.float32)
    ln_b = np.asarray(ln_b, np.float32)
    if "nc" not in _cached:
        _cached["nc"] = build_program()
    nc = _cached["nc"]
    in_maps = [_prep_core_inputs(h[:, b, :], W_qkvb, W_o, ln_g, ln_b)
               for b in range(BSZ)]
    res = run_bass_kernel_spmd(nc, in_maps, list(range(BSZ)),
                               trace=os.environ.get("BASS_TRACE", "") == "1")
    out = np.stack([res.results[b]["out"] for b in range(BSZ)], axis=1)
    kernel.last_exec_time_ns = res.exec_time_ns
    return out.astype(np.float32)


# revision 72
# speedup vs baseline: 1.1313x; 1.1313x over previous
"""Trainium2 Bass kernel for the CudaNorm FastWeight DPFP transformer layer.

Sharding: batch (8) across the 8 cores; each core runs its batch's full layer:
qkvb projection, DPFP feature maps, chunked delta-rule fast-weight scan
(C=128, depth-2 Neumann solve), output projection, residual + LayerNorm.

This revision batches the per-head elementwise work across all 16 heads with
strided (3D) access patterns, computes the attention denominator via tiny
column-sum matmuls on the (underutilized) PE instead of wide masked DVE
reductions, merges the Gram/S1 matmuls, and rebalances PSUM->SBUF copies onto
the Activation engine.  DVE instruction count per chunk drops ~4x.

Self-contained: hardcodes all shapes; host-side prep rearranges weights and
builds masks/identity constants passed as extra DRAM inputs.
"""
import os
import numpy as np
import ml_dtypes

import concourse.bass as bass
import concourse.mybir as mybir
from concourse.bass_utils import run_bass_kernel_spmd
from concourse.tile import TileContext
from concourse.vector_clock import ScopedClock, VectorClock
from contextlib import ExitStack

F32 = mybir.dt.float32
BF16 = mybir.dt.bfloat16
AF = mybir.ActivationFunctionType
OP = mybir.AluOpType
AX = mybir.AxisListType

SLEN, BSZ, DM = 2048, 8, 1024
NH, DH, NROLL = 16, 64, 2
D = 2 * NROLL * DH            # 256 feature dim
C = 128                       # chunk length
NCH = SLEN // C               # 16 chunks
EPS, LN_EPS = 1e-5, 1e-5
SCALE = 1.0 / float(np.sqrt(DH))
OQKV = NH * 192               # 3072
OTOT = OQKV + NH              # 3088 (qkv + per-head b columns)
BLK = 2 * 257                 # KQ per-head block: (dc) x [K 128 | r 1 | Q 128]
NEUMANN_DEPTH = int(os.environ.get("NEUMANN_DEPTH", "2"))

# ---------------------------------------------------------------- tile ctx
MAXW = 2


class PatchedTileContext(TileContext):
    """Work around walrus TPB sync-command limits: each instruction carries at
    most 2 sync commands (waits+updates); hoist excess waits onto preceding
    same-engine NoOps (1 wait each), and emit the kernel-tail drain's waits
    one-per-nop on SP."""

    def _lower_ordered_insts(self, ordered):
        for bb_name in list(ordered.keys()):
            new = []
            for inst in ordered[bb_name]:
                si = inst.sync_info
                nupd = len(si.on_update) if si is not None and si.on_update else 0
                maxw = max(0, MAXW - nupd)
                if si is not None and si.on_wait and len(si.on_wait) > maxw:
                    waits = list(si.on_wait)
                    excess = waits if maxw == 0 else waits[:-maxw]
                    keep = [] if maxw == 0 else waits[-maxw:]
                    for w in excess:
                        nop = mybir.InstNoOp(
                            name=self.nc.get_next_instruction_name(),
                            engine=inst.engine, ins=[], outs=[])
                        nop.sync_info = mybir.SyncInfo(on_wait=[w], on_update=[])
                        new.append(nop)
                    inst.sync_info = mybir.SyncInfo(
                        on_wait=keep, on_update=list(si.on_update or []))
                new.append(inst)
            ordered[bb_name] = new
        return super()._lower_ordered_insts(ordered)

    def _drain_and_barrier(self, tick_clock, wait_clock):
        gc = tick_clock.global_clock
        n = len(gc)
        for p in range(n):
            if gc[p] > 0:
                vc = VectorClock([gc[i] if i == p else 0 for i in range(n)])
                nop = self.nc.sync.nop(nofuse=True)
                wait_clock.add_sem_waits(nop.ins, ScopedClock({None: vc}))
        self.nc.sync.drain()
        self.nc.all_engine_barrier()
        assert self.sems is not None
        popped = self.nc._tile_sem_poison_stack.pop()
        assert popped is self._sem_poison
        self.nc.clear_and_free_semaphores(list(self.sems.allocated().values()))
        self.nc.all_engine_barrier()


# ---------------------------------------------------------------- program
def build_program(n_chunks=NCH, n_heads=NH):
    nc = bass.Bass()
    d_hT = nc.declare_dram_parameter("hT", [DM, SLEN], BF16, isOutput=False)
    d_hres = nc.declare_dram_parameter("hres", [SLEN, DM], F32, isOutput=False)
    d_w = nc.declare_dram_parameter("wqkv", [DM, OTOT], BF16, isOutput=False)
    d_wo = nc.declare_dram_parameter("woT", [DM, DM], BF16, isOutput=False)
    d_lng = nc.declare_dram_parameter("lng", [128, DM], BF16, isOutput=False)
    d_lnb = nc.declare_dram_parameter("lnb", [128, DM], BF16, isOutput=False)
    d_mSL = nc.declare_dram_parameter("maskSL", [128, 132], F32, isOutput=False)
    d_mUI = nc.declare_dram_parameter("maskUI", [128, 128], F32, isOutput=False)
    d_id = nc.declare_dram_parameter("identb", [128, 128], BF16, isOutput=False)
    d_out = nc.declare_dram_parameter("out", [SLEN, DM], F32, isOutput=True)

    with PatchedTileContext(nc) as tc, ExitStack() as ctx:
        P = lambda name, bufs, **kw: ctx.enter_context(
            tc.tile_pool(name=name, bufs=bufs, **kw))
        const = P("const", 1)
        state = P("state", 1)
        fr2 = P("fr2", 2)          # front-phase tiles also read in pass 2
        fr1 = P("fr1", 1)          # front-phase scratch (dead before pass 2)
        p2 = P("p2", 1)            # pass-2 tiles
        psA_p = P("psA", 3, space="PSUM")   # proj / pass1 / dn (tag blk)
        psT_p = P("psT", 2, space="PSUM")   # transpose groups (tag tp, bf16)
        psS_p = P("psS", 2, space="PSUM")   # solve/state groups (tag s)
        psO_p = P("psO", 1, space="PSUM")   # out-proj (pAT) + outT/rs (ptO)

        # ---- chunk-0 inputs first so their DMAs precede the weight bulk
        hts0 = fr2.tile([128, 1024], BF16, tag="hts", name="hts")
        nc.sync.dma_start(hts0.rearrange("p (mc t) -> p mc t", mc=8),
                          d_hT.rearrange("(mc p) t -> p mc t", p=128)[:, :, 0:128])
        hr0 = fr2.tile([128, DM], F32, tag="hr", name="hr")
        nc.sync.dma_start(hr0[:], d_hres[0:128, :])

        # ---- constants (weights first: og0 gates the chunk-0 projection)
        t_w = const.tile([128, 8 * OTOT], BF16, tag="w", name="w")
        for og in range(7):
            o0, ow = og * 512, (512 if og < 6 else NH)
            nc.sync.dma_start(
                t_w.rearrange("p (mc o) -> p mc o", mc=8)[:, :, o0:o0 + ow],
                d_w.rearrange("(mc p) o -> p mc o", p=128)[:, :, o0:o0 + ow])
        t_id = const.tile([128, 128], BF16, tag="id", name="id"); nc.sync.dma_start(t_id[:], d_id[:])
        t_mSL = const.tile([128, 132], F32, tag="mSL", name="mSL"); nc.sync.dma_start(t_mSL[:], d_mSL[:])
        t_mUI = const.tile([128, 128], F32, tag="mUI", name="mUI"); nc.sync.dma_start(t_mUI[:], d_mUI[:])
        t_lng = const.tile([128, DM], BF16, tag="lng", name="lng"); nc.sync.dma_start(t_lng[:], d_lng[:])
        t_lnb = const.tile([128, DM], BF16, tag="lnb", name="lnb"); nc.sync.dma_start(t_lnb[:], d_lnb[:])
        t_wo = const.tile([128, 8 * DM], BF16, tag="wo", name="wo")
        nc.sync.dma_start(t_wo.rearrange("p (ic o) -> p ic o", ic=8),
                          d_wo.rearrange("(ic p) o -> p ic o", p=128))
        t_ones = const.tile([128, 1], BF16, tag="ones", name="ones")
        nc.vector.memset(t_ones[:], 1.0)

        # ---- state: W [feat-in-dc 128, (h,dc) x 64], r [128, (h,dc)]
        t_Wm = state.tile([128, NH * 128], F32, tag="Wm", name="Wm")
        nc.vector.memset(t_Wm[:], 0.0)
        t_Wb = state.tile([128, NH * 128], BF16, tag="Wb", name="Wb")
        nc.vector.memset(t_Wb[:], 0.0)
        t_r = state.tile([128, 2 * NH], F32, tag="r", name="r")
        nc.vector.memset(t_r[:], 0.0)
        t_rb = state.tile([128, 2 * NH], BF16, tag="rb", name="rb")
        nc.vector.memset(t_rb[:], 0.0)

        def inject_r(nf):
            """Copy the (old) r state into KQ's per-(h,dc) column 128."""
            KQh = nf["KQ"].rearrange("p (h c) -> p h c", c=BLK)
            rbv = t_rb.rearrange("p (h d) -> p h d", d=2)
            for dc in range(2):
                nc.gpsimd.tensor_copy(KQh[:, :, dc * 257 + 128: dc * 257 + 129],
                                      rbv[:, :, dc:dc + 1])

        def front(c, hts=None, hr=None):
            cs = slice(c * 128, (c + 1) * 128)
            # ================= projection =================================
            if hts is None:
                hts = fr2.tile([128, 1024], BF16, tag="hts", name="hts")
                nc.sync.dma_start(hts.rearrange("p (mc t) -> p mc t", mc=8),
                                  d_hT.rearrange("(mc p) t -> p mc t", p=128)[:, :, cs])
                hr = fr2.tile([128, DM], F32, tag="hr", name="hr")
                nc.sync.dma_start(hr[:], d_hres[cs, :])
            raw = fr2.tile([128, 1024], BF16, tag="raw", name="raw")  # v only
            sig = fr2.tile([128, NH], F32, tag="sig", name="sig")
            # qk-first weight layout: cols h*128 = [q(h) 64 | k(h) 64] for
            # h<16, then v blocks, then b.  relu reads the projection PSUM
            # directly; relu+rolls split per og group so DVE front work
            # starts right after the first projection group.
            xp = fr1.tile([128, 2 * NH * 128], BF16, tag="xp", name="xp")
            xpv = xp.rearrange("p (b c) -> p b c", c=128)
            f_q = fr1.tile([128, NH * 256], BF16, tag="f_q", name="f_q")
            f_k = fr2.tile([128, NH * 256], BF16, tag="f_k", name="f_k")
            fs = fr1.tile([128, 32], F32, tag="fs", name="fs")
            frec = fr1.tile([128, 32], F32, tag="frec", name="frec")
            for og in range(7):
                o0, ow = og * 512, (512 if og < 6 else NH)
                pg = psA_p.tile([128, ow], F32, tag="blk", name="blk")
                for mc in range(8):
                    nc.tensor.matmul(pg[:], hts[:, mc * 128:(mc + 1) * 128],
                                     t_w[:, mc * OTOT + o0: mc * OTOT + o0 + ow],
                                     start=(mc == 0), stop=(mc == 7))
                if og < 4:
                    pgh = pg.rearrange("p (h c) -> p h c", c=128)
                    hs = slice(og * 4, og * 4 + 4)
                    nc.scalar.activation(xpv[:, hs, 0:64], pgh[:, :, 0:64], AF.Relu)
                    nc.scalar.activation(xpv[:, hs, 64:128], pgh[:, :, 0:64],
                                         AF.Relu, scale=-1.0)
                    ks = slice(16 + og * 4, 16 + og * 4 + 4)
                    nc.scalar.activation(xpv[:, ks, 0:64], pgh[:, :, 64:128], AF.Relu)
                    nc.scalar.activation(xpv[:, ks, 64:128], pgh[:, :, 64:128],
                                         AF.Relu, scale=-1.0)
                    for g, ft in ((0, f_q), (1, f_k)):
                        xv = xpv[:, g * 16 + og * 4: g * 16 + og * 4 + 4, :]
                        fv = ft.rearrange("p (h c) -> p h c",
                                          c=256)[:, og * 4: og * 4 + 4, :]
                        nc.vector.tensor_mul(fv[:, :, 1:128], xv[:, :, 1:128],
                                             xv[:, :, 0:127])
                        nc.vector.tensor_mul(fv[:, :, 0:1], xv[:, :, 0:1],
                                             xv[:, :, 127:128])
                        nc.vector.tensor_mul(fv[:, :, 130:256], xv[:, :, 2:128],
                                             xv[:, :, 0:126])
                        nc.vector.tensor_mul(fv[:, :, 128:130], xv[:, :, 0:2],
                                             xv[:, :, 126:128])
                elif og < 6:
                    nc.scalar.copy(raw[:, (og - 4) * 512:(og - 3) * 512], pg[:])
                else:
                    nc.scalar.activation(sig[:], pg[:], AF.Sigmoid)
            # normalize only the k features: the q normalization cancels in
            # out/denominator (both scale by 1/sum_q) up to the eps term
            fv = f_k.rearrange("p (h c) -> p h c", c=256)
            fold = fr1.tile([128, NH * 128], BF16, tag="fold", name="fold")
            foldv = fold.rearrange("p (h c) -> p h c", c=128)
            nc.vector.tensor_add(foldv[:], fv[:, :, 0:128], fv[:, :, 128:256])
            fold2 = fr1.tile([128, NH * 64], BF16, tag="fold2", name="fold2")
            f2v = fold2.rearrange("p (h c) -> p h c", c=64)
            nc.vector.tensor_add(f2v[:], foldv[:, :, 0:64], foldv[:, :, 64:128])
            nc.vector.tensor_reduce(fs[:, 0:16], f2v[:], AX.X, OP.add)
            nc.vector.reciprocal(frec[:, 0:16], fs[:, 0:16])
            for hd in range(n_heads):
                sl = f_k[:, hd * 256:(hd + 1) * 256]
                if hd % 2 == 0:
                    nc.scalar.mul(sl, sl, frec[:, hd: hd + 1])
                else:
                    nc.vector.tensor_scalar_mul(sl, sl, frec[:, hd: hd + 1])

            # ================= KQ: transposed features ====================
            # KQ per (h,dc) 257-block: [K^T 128 | r 1 | Q^T 128]; the r column
            # is injected by back(c-1) after its r-state update.
            KQ = fr2.tile([128, NH * BLK], BF16, tag="KQ", name="KQ")
            KQh = KQ.rearrange("p (h c) -> p h c", c=BLK)
            for g, ft in ((0, f_q), (1, f_k)):
                for dc in range(2):
                    for hg in range(2):
                        pt = psT_p.tile([128, 1024], BF16, tag="tp", name="tp")
                        for j in range(8):
                            hd = hg * 8 + j
                            nc.tensor.transpose(
                                pt[:, j * 128:(j + 1) * 128],
                                ft[:, hd * 256 + dc * 128: hd * 256 + dc * 128 + 128],
                                t_id[:])
                        off = dc * 257 + (129 if g == 0 else 0)
                        nc.scalar.copy(
                            KQh[:, hg * 8:(hg + 1) * 8, off:off + 128],
                            pt.rearrange("p (j c) -> p j c", j=8))
            return {"hr": hr, "raw": raw, "sig": sig, "f_k": f_k, "KQ": KQ}

        def back(c, cur, nxt):
            cs = slice(c * 128, (c + 1) * 128)
            raw, sig, f_k, KQ, hr = (cur["raw"], cur["sig"], cur["f_k"],
                                     cur["KQ"], cur["hr"])
            # ================= pass 1: Gram+S1, kd, sh =====================
            kd_all = p2.tile([128, NH], F32, tag="kd", name="kd")
            Am = p2.tile([128, NH * 132], BF16, tag="Am", name="Am")
            sh = p2.tile([128, NH * 128], BF16, tag="sh", name="sh")
            for hd in range(n_heads):
                pAS = psA_p.tile([128, 260], F32, tag="blk", name="blk")
                for dc in range(2):
                    b = hd * BLK + dc * 257
                    nc.tensor.matmul(pAS[:, 0:257], KQ[:, b:b + 128],
                                     KQ[:, b:b + 257],
                                     start=(dc == 0), stop=(dc == 1))
                nc.vector.scalar_tensor_tensor(
                    Am[:, hd * 132: hd * 132 + 129], pAS[:, 0:129], 1.0,
                    t_mSL[:, 0:129], OP.mult, OP.mult,
                    accum_out=kd_all[:, hd:hd + 1])
                nc.vector.tensor_mul(sh[:, hd * 128:(hd + 1) * 128],
                                     pAS[:, 129:257], t_mUI[:])
            if c == 0:
                nc.vector.memset(kd_all[0:1, :], 1.0)

            # dn[t] = colsum(sh)[t] + q_t . r   (tiny PE matmuls)
            p_dn = psA_p.tile([128, NH], F32, tag="blk", name="blk")
            for hd in range(n_heads):
                nc.tensor.matmul(p_dn[:, hd:hd + 1], sh[:, hd * 128:(hd + 1) * 128],
                                 t_ones[:], start=True, stop=False)
                nc.tensor.matmul(p_dn[:, hd:hd + 1],
                                 KQ[:, hd * BLK + 129: hd * BLK + 257],
                                 t_rb[:, hd * 2: hd * 2 + 1], start=False, stop=False)
                nc.tensor.matmul(p_dn[:, hd:hd + 1],
                                 KQ[:, hd * BLK + 257 + 129: hd * BLK + 257 + 257],
                                 t_rb[:, hd * 2 + 1: hd * 2 + 2], start=False, stop=True)
            dn_all = p2.tile([128, NH], F32, tag="dn", name="dn")
            nc.scalar.copy(dn_all[:], p_dn[:])

            # ================= chunk-level columns =========================
            ceps = p2.tile([128, NH], F32, tag="ceps", name="ceps")
            nc.vector.tensor_scalar_add(ceps[:], kd_all[:], EPS)
            c_all = p2.tile([128, NH], F32, tag="c", name="c")
            nc.vector.reciprocal(c_all[:], ceps[:])
            t0 = p2.tile([128, NH], F32, tag="t0", name="t0")
            nc.vector.tensor_mul(t0[:], kd_all[:], c_all[:])
            cb_all = p2.tile([128, NH], F32, tag="cb", name="cb")
            nc.vector.tensor_mul(cb_all[:], t0[:], sig[:])
            cbc = p2.tile([128, NH], F32, tag="cbc", name="cbc")
            nc.vector.tensor_mul(cbc[:], cb_all[:], c_all[:])
            ncbc = p2.tile([128, NH], F32, tag="ncbc", name="ncbc")
            nc.vector.tensor_scalar_mul(ncbc[:], cbc[:], -1.0)
            dne = p2.tile([128, NH], F32, tag="dne", name="dne")
            nc.vector.tensor_scalar_add(dne[:], dn_all[:], EPS)
            dnr = p2.tile([128, NH], F32, tag="dnr", name="dnr")
            nc.vector.reciprocal(dnr[:], dne[:])
            dnrS = p2.tile([128, NH], F32, tag="dnrS", name="dnrS")
            nc.vector.tensor_scalar_mul(dnrS[:], dnr[:], SCALE)

            # ================= pass 2 (step-major over heads) ==============
            # N'' = masked Gram * cbc (scaled in place), its transpose Bt
            for hd in range(n_heads):
                nc.vector.tensor_scalar_mul(Am[:, hd * 132: hd * 132 + 128],
                                            Am[:, hd * 132: hd * 132 + 128],
                                            cbc[:, hd:hd + 1])
            Bt = p2.tile([128, NH * 128], BF16, tag="Bt", name="Bt")
            for hg in range(2):
                pt = psT_p.tile([128, 1024], BF16, tag="tp", name="tp")
                for j in range(8):
                    hd = hg * 8 + j
                    nc.tensor.transpose(pt[:, j * 128:(j + 1) * 128],
                                        Am[:, hd * 132: hd * 132 + 128], t_id[:])
                nc.scalar.copy(Bt[:, hg * 1024:(hg + 1) * 1024], pt[:])

            # cbV = cb * V ; X0 = cbV - cbc * (K W)
            cbV = p2.tile([128, NH * DH], BF16, tag="cbV", name="cbV")
            for hd in range(n_heads):
                nc.scalar.mul(cbV[:, hd * 64:(hd + 1) * 64],
                              raw[:, hd * 64:(hd + 1) * 64],
                              cb_all[:, hd:hd + 1])
            if c == 0:
                X0 = cbV
            else:
                X0 = p2.tile([128, NH * DH], BF16, tag="X0", name="X0")
                pKW = []
                for hg in range(2):
                    pk = psS_p.tile([128, 512], F32, tag="s", name="s")
                    for j in range(8):
                        hd = hg * 8 + j
                        for dc in range(2):
                            nc.tensor.matmul(
                                pk[:, j * 64:(j + 1) * 64],
                                KQ[:, hd * BLK + dc * 257: hd * BLK + dc * 257 + 128],
                                t_Wb[:, hd * 128 + dc * 64: hd * 128 + dc * 64 + 64],
                                start=(dc == 0), stop=(dc == 1))
                    pKW.append(pk)
                for hg in range(2):
                    for j in range(8):
                        hd = hg * 8 + j
                        nc.vector.scalar_tensor_tensor(
                            X0[:, hd * 64:(hd + 1) * 64],
                            pKW[hg][:, j * 64:(j + 1) * 64],
                            ncbc[:, hd:hd + 1],
                            cbV[:, hd * 64:(hd + 1) * 64], OP.mult, OP.add)

            # Neumann solve: depth2: Y = X0 - N(X0 - N X0)
            def mmN(dst_pool_tag, rhs, hg):
                ps = psS_p.tile([128, 512], F32, tag="s", name="s")
                for j in range(8):
                    hd = hg * 8 + j
                    nc.tensor.matmul(ps[:, j * 64:(j + 1) * 64],
                                     Bt[:, hd * 128:(hd + 1) * 128],
                                     rhs[:, hd * 64:(hd + 1) * 64],
                                     start=True, stop=True)
                return ps

            if NEUMANN_DEPTH == 2:
                X1 = p2.tile([128, NH * DH], BF16, tag="X1", name="X1")
                for hg in range(2):
                    pX = mmN("s", X0, hg)
                    nc.vector.tensor_sub(X1[:, hg * 512:(hg + 1) * 512],
                                         X0[:, hg * 512:(hg + 1) * 512], pX[:])
                Yt = X0  # overwritten in place; X0 is dead after the subs
                for hg in range(2):
                    pY = mmN("s", X1, hg)
                    nc.vector.tensor_sub(Yt[:, hg * 512:(hg + 1) * 512],
                                         X0[:, hg * 512:(hg + 1) * 512], pY[:])
            else:
                # depth-3 (baseline): B2 = N N^T-form, X1 = X0 + N^2 X0,
                # Y = X1 - N X1
                B2 = p2.tile([128, NH * 128], BF16, tag="B2", name="B2")
                for hg in range(2):
                    pb2 = psS_p.tile([128, 1024], F32, tag="s2", name="s2")
                    for j in range(8):
                        hd = hg * 8 + j
                        nc.tensor.matmul(pb2[:, j * 128:(j + 1) * 128],
                                         Nt[:, hd * 128:(hd + 1) * 128],
                                         Bt[:, hd * 128:(hd + 1) * 128],
                                         start=True, stop=True)
                    nc.scalar.copy(B2[:, hg * 1024:(hg + 1) * 1024], pb2[:])
                X1 = p2.tile([128, NH * DH], BF16, tag="X1", name="X1")
                for hg in range(2):
                    ps = psS_p.tile([128, 512], F32, tag="s", name="s")
                    for j in range(8):
                        hd = hg * 8 + j
                        nc.tensor.matmul(ps[:, j * 64:(j + 1) * 64],
                                         B2[:, hd * 128:(hd + 1) * 128],
                                         X0[:, hd * 64:(hd + 1) * 64],
                                         start=True, stop=True)
                    nc.vector.tensor_add(X1[:, hg * 512:(hg + 1) * 512],
                                         X0[:, hg * 512:(hg + 1) * 512], ps[:])
                Yt = p2.tile([128, NH * DH], BF16, tag="Yt", name="Yt")
                for hg in range(2):
                    pY = mmN("s", X1, hg)
                    nc.vector.tensor_sub(Yt[:, hg * 512:(hg + 1) * 512],
                                         X1[:, hg * 512:(hg + 1) * 512], pY[:])

            # Out = QW + tril(S1) Y, scaled by SCALE/(denom+eps); transpose
            outT = p2.tile([128, NH * DH], BF16, tag="outT", name="outT")
            ptO = psT_p.tile([128, 1024], BF16, tag="tp", name="tp")
            for hg in range(2):
                po = psS_p.tile([128, 512], F32, tag="s", name="s")
                for j in range(8):
                    hd = hg * 8 + j
                    o = slice(j * 64, (j + 1) * 64)
                    if c > 0:
                        for dc in range(2):
                            nc.tensor.matmul(
                                po[:, o],
                                KQ[:, hd * BLK + dc * 257 + 129: hd * BLK + dc * 257 + 257],
                                t_Wb[:, hd * 128 + dc * 64: hd * 128 + dc * 64 + 64],
                                start=(dc == 0), stop=False)
                    nc.tensor.matmul(po[:, o], sh[:, hd * 128:(hd + 1) * 128],
                                     Yt[:, hd * 64:(hd + 1) * 64],
                                     start=(c == 0), stop=True)
                outc = p2.tile([128, 512], BF16, tag=f"outc{hg}", name=f"outc{hg}")
                for j in range(8):
                    hd = hg * 8 + j
                    nc.scalar.mul(outc[:, j * 64:(j + 1) * 64],
                                  po[:, j * 64:(j + 1) * 64], dnrS[:, hd:hd + 1])
                for j in range(8):
                    hd = hg * 8 + j
                    base = (hd % 2) * 64
                    ic = hd // 2
                    nc.tensor.transpose(ptO[base:base + 64, ic * 128:(ic + 1) * 128],
                                        outc[:, j * 64:(j + 1) * 64], t_id[:],
                                        tile_position=(0, base))
            nc.scalar.copy(outT[:], ptO[:])

            # W update: W += K^T Y (per (hg, dc) psum groups), r update
            Wmv = t_Wm.rearrange("p (h c) -> p h c", c=128)
            for hg in range(2):
                for dc in range(2):
                    pw = psS_p.tile([128, 512], F32, tag="s", name="s")
                    for j in range(8):
                        hd = hg * 8 + j
                        nc.tensor.matmul(pw[:, j * 64:(j + 1) * 64],
                                         f_k[:, hd * 256 + dc * 128: hd * 256 + dc * 128 + 128],
                                         Yt[:, hd * 64:(hd + 1) * 64],
                                         start=True, stop=True)
                    wv = Wmv[:, hg * 8:(hg + 1) * 8, dc * 64:(dc + 1) * 64]
                    nc.vector.tensor_add(wv, pw.rearrange("p (j c) -> p j c", j=8), wv)
                    nc.gpsimd.tensor_copy(
                        t_Wb.rearrange("p (h c) -> p h c",
                                       c=128)[:, hg * 8:(hg + 1) * 8,
                                              dc * 64:(dc + 1) * 64], wv)
            # r += per-(h,dc) row-sums of K over time, via ones-matmuls
            p_rs = psO_p.tile([128, 2 * NH], F32, tag="pAT", name="prs")
            for b in range(2 * NH):
                nc.tensor.matmul(p_rs[:, b:b + 1],
                                 f_k[:, b * 128:(b + 1) * 128],
                                 t_ones[:], start=True, stop=True)
            nc.vector.tensor_add(t_r[:], t_r[:], p_rs[:])
            nc.gpsimd.tensor_copy(t_rb[:], t_r[:])
            if nxt is not None:
                inject_r(nxt)

            # ================= output projection + residual + LN ============
            x = p2.tile([128, DM], F32, tag="x", name="x")
            for og in range(2):
                pAT = psO_p.tile([128, 512], F32, tag="pAT", name="pAT")
                for ic in range(8):
                    nc.tensor.matmul(pAT[:], outT[:, ic * 128:(ic + 1) * 128],
                                     t_wo[:, ic * DM + og * 512: ic * DM + og * 512 + 512],
                                     start=(ic == 0), stop=(ic == 7))
                nc.vector.tensor_add(x[:, og * 512:(og + 1) * 512], pAT[:],
                                     hr[:, og * 512:(og + 1) * 512])
            xsum = p2.tile([128, 1], F32, tag="xsum", name="xsum")
            nc.scalar.activation(hr[:], x[:], AF.Copy, accum_out=xsum[:])
            nmu = p2.tile([128, 1], F32, tag="nmu", name="nmu")
            nc.vector.tensor_scalar_mul(nmu[:], xsum[:], -1.0 / DM)
            nc.vector.tensor_scalar_add(x[:], x[:], nmu[:])
            var = p2.tile([128, 1], F32, tag="var", name="var")
            nc.scalar.activation(hr[:], x[:], AF.Square, accum_out=var[:])
            vare = p2.tile([128, 1], F32, tag="vare", name="vare")
            nc.vector.tensor_scalar(vare[:], var[:], 1.0 / DM, float(LN_EPS),
                                    OP.mult, OP.add)
            sd = p2.tile([128, 1], F32, tag="sd", name="sd")
            nc.scalar.sqrt(sd[:], vare[:])
            rstd = p2.tile([128, 1], F32, tag="rstd", name="rstd")
            nc.vector.reciprocal(rstd[:], sd[:])
            nc.vector.scalar_tensor_tensor(x[:], x[:], rstd[:], t_lng[:],
                                           OP.mult, OP.mult)
            nc.vector.tensor_add(hr[:], x[:], t_lnb[:])
            nc.sync.dma_start(d_out[cs, :], hr[:])

        # software pipeline: emit front(c+1) ahead of back(c) so the
        # scheduler fills back(c)'s dependency stalls with front work
        tiles = {0: front(0, hts0, hr0)}
        inject_r(tiles[0])
        for c in range(n_chunks):
            if c + 1 < n_chunks:
                tiles[c + 1] = front(c + 1)
            back(c, tiles.pop(c), tiles.get(c + 1))

    return nc


# ---------------------------------------------------------------- host side
def _prep_core_inputs(h_b, W_qkvb, W_o, ln_g, ln_b):
    bf16 = ml_dtypes.bfloat16
    hT = np.ascontiguousarray(h_b.T).astype(bf16)                  # [1024, 2048]
    wq = np.zeros((DM, OTOT), dtype=bf16)
    Wr = W_qkvb.reshape(NH, 193, DM)
    for hd in range(NH):
        wq[:, hd * 128:hd * 128 + 64] = Wr[hd, 0:64, :].T        # q
        wq[:, hd * 128 + 64:hd * 128 + 128] = Wr[hd, 64:128, :].T  # k
        wq[:, 2048 + hd * 64:2048 + hd * 64 + 64] = Wr[hd, 128:192, :].T  # v
        wq[:, OQKV + hd] = Wr[hd, 192, :]
    woT = np.ascontiguousarray(W_o.T).astype(bf16)                 # [i, o]
    lng = np.broadcast_to(ln_g[None, :], (128, DM)).astype(bf16).copy()
    lnb = np.broadcast_to(ln_b[None, :], (128, DM)).astype(bf16).copy()
    ii, jj = np.indices((128, 132))
    mSL = (jj < ii).astype(np.float32);  mSL[:, 128] = 1.0
    iu, ju = np.indices((128, 128))
    mUI = (ju >= iu).astype(np.float32)
    identb = np.eye(128, dtype=bf16)
    return {"hT": hT, "hres": np.ascontiguousarray(h_b, np.float32),
            "wqkv": wq, "woT": woT, "lng": lng, "lnb": lnb,
            "maskSL": mSL, "maskUI": mUI, "identb": identb}


_cached = {}


def kernel(h, W_qkvb, W_o, ln_g, ln_b):
    h = np.asarray(h, np.float32)
    W_qkvb = np.asarray(W_qkvb, np.float32)
    W_o = np.asarray(W_o, np.float32)
    ln_g = np.asarray(ln_g, np.float32)
    ln_b = np.asarray(ln_b, np.float32)
    if "nc" not in _cached:
        _cached["nc"] = build_program()
    nc = _cached["nc"]
    in_maps = [_prep_core_inputs(h[:, b, :], W_qkvb, W_o, ln_g, ln_b)
               for b in range(BSZ)]
    res = run_bass_kernel_spmd(nc, in_maps, list(range(BSZ)),
                               trace=os.environ.get("BASS_TRACE", "") == "1")
    out = np.stack([res.results[b]["out"] for b in range(BSZ)], axis=1)
    kernel.last_exec_time_ns = res.exec_time_ns
    return out.astype(np.float32)
